# revision 17
# baseline (speedup 1.0000x reference)
"""GAT 2-layer (PyG GATConv) model on 8 Trainium2 NeuronCores.

Strategy (graph/data parallel, dst-partitioned):
  * Nodes are sorted by in-degree (desc) and dealt round-robin to the 8
    cores, so block b on every core holds nodes of similar degree.
    Each core owns NPPC local node slots, processed in blocks of 128
    (one SBUF partition lane per destination node).
  * Edges are grouped by destination on the host.  For each block the
    destination node on lane p owns a run of "slots" in the free
    dimension.  One `dma_gather` (InstDMAGatherAnt) per block per table
    half pulls the per-edge source rows [h | a_src] from an AllGather'ed
    node table straight into the [128, S * W] slot grid (the int16 index
    limit forces a lo/hi table split; each lane's slots are split into a
    lo run and a hi run).  Padding slots point at a sentinel table row
    whose a_src = -6e4, which makes exp(leaky_relu(...)) == 0, so pads
    contribute nothing to messages or softmax denominators.
  * The segment softmax + weighted aggregation is dense per-block work:
    alpha = a_src + a_dst (a_dst is resident per-lane), leaky-relu
    (max(x, 0.2x)), exp on the scalar engine, message scale, and one
    strided reduce over the slot axis which also sums the denominators.
  * Layer 1 -> ELU -> layer-2 dense projection happen in the same block
    loop; a second AllGather publishes the layer-2 table; a second edge
    phase produces the output.  Layer-1 table is bf16 (rows padded to
    256B, the dma_gather element granularity); layer-2 table is f32.
"""

import ml_dtypes
import numpy as np

import concourse.bacc as bacc
import concourse.mybir as mybir
import concourse.tile as tile
from concourse.bass_utils import run_bass_kernel_spmd

F32 = mybir.dt.float32
BF16 = mybir.dt.bfloat16
I16 = mybir.dt.int16
AX = mybir.AxisListType.X
OP = mybir.AluOpType
EXP = mybir.ActivationFunctionType.Exp

NC = 8          # cores
P = 128         # partitions / nodes per block
HALF = 32768    # int16 index limit -> lo/hi table split
NEG_SLOPE = 0.2
PAD_ASRC = -60000.0   # sentinel a_src for padding slots: exp(leaky(.)) == 0

LAST_RESULTS = None   # stashed BassKernelResults for test harnesses
TRACE = False         # set True (e.g. from test.py) to capture an NTFF profile


def _ceil_to(x, m):
    return (x + m - 1) // m * m


def _sent_mask(w):
    m = np.zeros((P, w), np.float32)
    m[P - 1, :] = PAD_ASRC
    return m


def _wrap_idx(seg):
    """[NC, 128*S] position-major int16 -> ucode layout [NC, 128, 8*S]
    (idx i at partition i%16, column i//16; replicated across the 8
    16-partition groups)."""
    ncs, n = seg.shape
    w = seg.reshape(ncs, n // 16, 16).transpose(0, 2, 1)   # [NC, 16, cols]
    return np.tile(w, (1, 8, 1)).astype(np.int16)


# --------------------------------------------------------------------------
# host-side graph preprocessing
# --------------------------------------------------------------------------
def _preprocess(x, edge_index):
    N = x.shape[0]
    src = np.concatenate([np.asarray(edge_index[0]), np.arange(N, dtype=np.int64)])
    dst = np.concatenate([np.asarray(edge_index[1]), np.arange(N, dtype=np.int64)])
    src = src.astype(np.int64)
    dst = dst.astype(np.int64)

    deg = np.bincount(dst, minlength=N)
    order = np.argsort(-deg, kind="stable")          # rank -> node id
    rank = np.empty(N, dtype=np.int64)
    rank[order] = np.arange(N)

    core_of = rank % NC
    lid_of = rank // NC                              # local id on its core
    nppc = _ceil_to((N + NC - 1) // NC, P)           # local slots per core
    if nppc * NC <= N:                               # ensure a dummy lane exists
        nppc += P                                    # (hosts the pad sentinel)
    nb = nppc // P                                   # blocks per core
    R = NC * nppc

    gsid = core_of * nppc + lid_of                   # node -> table row
    assert not np.any((core_of == 0) & (lid_of == nppc - 1))
    SENT_LO = nppc - 1                               # core 0's last (dummy) lane
    HI_BASE = HALF if R > HALF else 0                # hi half empty if R fits
    SENT_HI = R - 1 - HI_BASE                        # core NC-1's last lane
    assert R - HI_BASE <= HALF and SENT_HI >= 0

    # per-edge half split and slot position within (node, half)
    gs = gsid[src]
    is_hi = (gs >= HI_BASE).astype(np.int64) if HI_BASE else np.zeros_like(gs)
    key = rank[dst] * 2 + is_hi
    eord = np.argsort(key, kind="stable")
    key_s = key[eord]
    gs_s = gs[eord]
    counts_k = np.bincount(key, minlength=2 * N)
    starts_k = np.concatenate([[0], np.cumsum(counts_k)])[:-1]
    slot = np.arange(key_s.shape[0], dtype=np.int64) - starts_k[key_s]

    k_lo = counts_k[0::2]                            # per rank
    k_hi = counts_k[1::2]
    nwin = nb * NC * P

    def _blockmax(a):
        pad = np.zeros(nwin, dtype=np.int64)
        pad[:N] = a
        return np.maximum(pad.reshape(nb, NC * P).max(1), 1)

    S_lo = _blockmax(k_lo)
    S_hi = _blockmax(k_hi)
    SBlo = np.concatenate([[0], np.cumsum(S_lo)]).astype(np.int64)
    SBhi = np.concatenate([[0], np.cumsum(S_hi)]).astype(np.int64)

    r_s = key_s // 2
    half_s = key_s & 1
    c_e = r_s % NC
    l_e = r_s // NC
    b_e = l_e // P
    p_e = l_e % P
    # position-major flat layout per half: pos = 128*SB[b] + slot*128 + lane
    pos_lo = P * SBlo[b_e] + slot * P + p_e
    pos_hi = P * SBhi[b_e] + slot * P + p_e

    G_lo = np.full((NC, int(P * SBlo[-1])), SENT_LO, dtype=np.int64)
    G_hi = np.full((NC, int(P * SBhi[-1])), SENT_HI, dtype=np.int64)
    lo = half_s == 0
    hi = ~lo
    G_lo[c_e[lo], pos_lo[lo]] = gs_s[lo]
    G_hi[c_e[hi], pos_hi[hi]] = gs_s[hi] - HI_BASE

    gilo = _wrap_idx(G_lo)                            # [NC, 128, 8*sum(S_lo)]
    gihi = _wrap_idx(G_hi)

    xg = np.zeros((NC, nppc, x.shape[1]), dtype=np.float32)
    xg[core_of, lid_of] = np.asarray(x, dtype=np.float32)

    return dict(N=N, nppc=nppc, nb=nb,
                S_lo=[int(v) for v in S_lo], S_hi=[int(v) for v in S_hi],
                SBlo=SBlo, SBhi=SBhi, gilo=gilo, gihi=gihi, HI_BASE=HI_BASE,
                xg=xg, order=order)



GMAX_COLS = 8     # <=1024 indices per dma_gather call (SWDGE ring cap)


def _emit_gathers(nc, et_ap, tw, tbl_lo_ap, tbl_hi_ap, gxl, gxh, Slo, Shi, qctr):
    """Emit capped dma_gather calls filling slot columns [0,Slo+Shi) of the
    block's slot grid. Returns updated queue counter."""
    for half, (tbl_ap, gx, scnt, base) in enumerate(
            ((tbl_lo_ap, gxl, Slo, 0), (tbl_hi_ap, gxh, Shi, Slo))):
        done = 0
        while done < scnt:
            take = min(GMAX_COLS, scnt - done)
            nc.gpsimd.dma_gather(
                out_ap=et_ap[:, (base + done) * tw:(base + done + take) * tw]
                    .rearrange("p (s w) -> p s w", w=tw),
                in_ap=tbl_ap,
                idxs_ap=gx[:, 8 * done:8 * (done + take)],
                num_idxs=P * take, num_idxs_reg=P * take, elem_size=tw,
                queue_num=0)
            qctr += 1
            done += take
    return qctr


# --------------------------------------------------------------------------
# device program
# --------------------------------------------------------------------------
def _build_program(meta, IN_CH, HEADS, HID, OUT_CH, debug_outs=False):
    F1 = HEADS * HID                                  # 64
    TW1 = 2 * F1                                      # 128 bf16 = 256B rows
    TW2 = F1                                          # 64 f32  = 256B rows
    nppc, nb = meta["nppc"], meta["nb"]
    S_lo, S_hi, SBlo, SBhi = (meta["S_lo"], meta["S_hi"],
                              meta["SBlo"], meta["SBhi"])
    R = NC * nppc
    HB = meta["HI_BASE"]
    LO_END = min(HALF, R)
    KC = IN_CH // P
    CL = 8 * int(SBlo[-1])                            # idx columns, lo
    CH = 8 * int(SBhi[-1])

    nc = bacc.Bacc("TRN2", target_bir_lowering=False, debug=False,
                   enable_asserts=False, num_devices=NC)

    xg_d = nc.dram_tensor("xg", [nppc, IN_CH], F32, kind="ExternalInput").ap()
    w1_d = nc.dram_tensor("w1", [IN_CH, F1], F32, kind="ExternalInput").ap()
    as1_d = nc.dram_tensor("as1", [P, F1], F32, kind="ExternalInput").ap()
    ad1_d = nc.dram_tensor("ad1", [P, F1], F32, kind="ExternalInput").ap()
    b1_d = nc.dram_tensor("b1r", [P, F1], F32, kind="ExternalInput").ap()
    w2_d = nc.dram_tensor("w2", [F1, OUT_CH], F32, kind="ExternalInput").ap()
    as2_d = nc.dram_tensor("as2", [P, OUT_CH], F32, kind="ExternalInput").ap()
    ad2_d = nc.dram_tensor("ad2", [P, OUT_CH], F32, kind="ExternalInput").ap()
    b2_d = nc.dram_tensor("b2r", [P, OUT_CH], F32, kind="ExternalInput").ap()
    id_d = nc.dram_tensor("ident", [P, P], F32, kind="ExternalInput").ap()
    glo_d = nc.dram_tensor("gilo", [P, CL], I16, kind="ExternalInput").ap()
    ghi_d = nc.dram_tensor("gihi", [P, CH], I16, kind="ExternalInput").ap()
    sm1_d = nc.dram_tensor("smask1", [P, HEADS], F32, kind="ExternalInput").ap()
    sm2_d = nc.dram_tensor("smask2", [P, 1], F32, kind="ExternalInput").ap()
    out_d = nc.dram_tensor("out", [nppc, OUT_CH], F32, kind="ExternalOutput").ap()
    if debug_outs:
        dden1_d = nc.dram_tensor("dden1", [nppc, HEADS], F32, kind="ExternalOutput").ap()

    rg = [list(range(NC))]

    with tile.TileContext(nc) as tc:
        with (tc.tile_pool(name="const", bufs=1) as cp,
              tc.tile_pool(name="work", bufs=3) as wp,
              tc.tile_pool(name="edge", bufs=2) as ep,
              tc.tile_pool(name="psum", bufs=2, space="PSUM") as pp,
              tc.tile_pool(name="dram", bufs=1, space="DRAM") as dp):

            # ---- constants -------------------------------------------------
            w1sb = cp.tile([P, KC * F1], F32)
            nc.sync.dma_start(out=w1sb[:].rearrange("p (i j) -> p i j", j=F1),
                              in_=w1_d.rearrange("(i p) j -> p i j", p=P))
            idsb = cp.tile([P, P], F32)
            nc.sync.dma_start(out=idsb[:], in_=id_d)
            as1sb = cp.tile([P, F1], F32)
            nc.sync.dma_start(out=as1sb[:], in_=as1_d)
            ad1sb = cp.tile([P, F1], F32)
            nc.sync.dma_start(out=ad1sb[:], in_=ad1_d)
            b1sb = cp.tile([P, F1], F32)
            nc.sync.dma_start(out=b1sb[:], in_=b1_d)
            w2sb = cp.tile([F1, OUT_CH], F32)
            nc.sync.dma_start(out=w2sb[:], in_=w2_d)
            as2sb = cp.tile([P, OUT_CH], F32)
            nc.sync.dma_start(out=as2sb[:], in_=as2_d)
            ad2sb = cp.tile([P, OUT_CH], F32)
            nc.sync.dma_start(out=ad2sb[:], in_=ad2_d)
            b2sb = cp.tile([P, OUT_CH], F32)
            nc.sync.dma_start(out=b2sb[:], in_=b2_d)
            sm1sb = cp.tile([P, HEADS], F32)
            nc.sync.dma_start(out=sm1sb[:], in_=sm1_d)
            sm2sb = cp.tile([P, 1], F32)
            nc.sync.dma_start(out=sm2sb[:], in_=sm2_d)

            adst1 = cp.tile([P, nb * HEADS], F32)     # per-block a_dst, layer 1
            adst2 = cp.tile([P, nb], F32)             # per-block a_dst, layer 2

            tbl1_loc = dp.tile([nppc, TW1], BF16)
            tbl1 = dp.tile([R, TW1], BF16)
            tbl2_loc = dp.tile([nppc, TW2], F32)
            tbl2 = dp.tile([R, TW2], F32)

            # ---- phase A: dense layer-1 projection -------------------------
            for b in range(nb):
                r0 = b * P
                xt = wp.tile([P, IN_CH], F32, tag="xt")
                nc.sync.dma_start(out=xt[:], in_=xg_d[r0:r0 + P, :])
                rs = wp.tile([P, 1], F32, tag="rs")
                nc.vector.reduce_sum(out=rs[:], in_=xt[:], axis=AX)
                nc.vector.tensor_scalar_max(out=rs[:], in0=rs[:], scalar1=1e-8)
                rcp = wp.tile([P, 1], F32, tag="rcp")
                nc.vector.reciprocal(out=rcp[:], in_=rs[:])
                nc.vector.tensor_scalar_mul(out=xt[:], in0=xt[:], scalar1=rcp[:])

                hT = wp.tile([P, IN_CH], F32, tag="hT")
                for i in range(KC):
                    tp = pp.tile([P, P], F32, tag="tp")
                    nc.tensor.transpose(out=tp[:], in_=xt[:, i * P:(i + 1) * P],
                                        identity=idsb[:])
                    nc.scalar.copy(out=hT[:, i * P:(i + 1) * P], in_=tp[:])
                h1p = pp.tile([P, F1], F32, tag="mm1")
                for i in range(KC):
                    nc.tensor.matmul(out=h1p[:], lhsT=hT[:, i * P:(i + 1) * P],
                                     rhs=w1sb[:, i * F1:(i + 1) * F1],
                                     start=(i == 0), stop=(i == KC - 1))

                atmp = wp.tile([P, F1], F32, tag="atmp")
                asr = wp.tile([P, HEADS], F32, tag="asr")
                nc.vector.tensor_mul(out=atmp[:], in0=h1p[:], in1=as1sb[:])
                nc.vector.reduce_sum(
                    out=asr[:],
                    in_=atmp[:].rearrange("p (h c) -> p h c", c=HID), axis=AX)
                nc.vector.tensor_mul(out=atmp[:], in0=h1p[:], in1=ad1sb[:])
                nc.vector.reduce_sum(
                    out=adst1[:, b * HEADS:(b + 1) * HEADS],
                    in_=atmp[:].rearrange("p (h c) -> p h c", c=HID), axis=AX)
                if b == nb - 1:
                    # pad-slot sentinel rows ride the AllGather (last dummy
                    # lane of every core; smask1 = PAD_ASRC on lane P-1 only)
                    nc.vector.tensor_add(out=asr[:], in0=asr[:], in1=sm1sb[:])
                trow = wp.tile([P, TW1], BF16, tag="trow")
                nc.scalar.copy(out=trow[:, 0:F1], in_=h1p[:])
                nc.scalar.copy(out=trow[:, F1:F1 + HEADS], in_=asr[:])
                nc.vector.memset(trow[:, F1 + HEADS:TW1], 0.0)
                nc.sync.dma_start(out=tbl1_loc[r0:r0 + P, :], in_=trow[:])

            nc.gpsimd.collective_compute(
                "AllGather", OP.bypass, replica_groups=rg,
                ins=[tbl1_loc[:].opt()], outs=[tbl1[:].opt()])

            # ---- phase B: edge layer 1 + dense layer 2 ---------------------
            qctr = 0
            for b in range(nb):
                r0 = b * P
                Slo, Shi = S_lo[b], S_hi[b]
                S = Slo + Shi
                gxl = ep.tile([P, 8 * Slo], I16, tag="gxl")
                nc.sync.dma_start(out=gxl[:],
                                  in_=glo_d[:, 8 * int(SBlo[b]):8 * int(SBlo[b]) + 8 * Slo])
                gxh = ep.tile([P, 8 * Shi], I16, tag="gxh")
                nc.sync.dma_start(out=gxh[:],
                                  in_=ghi_d[:, 8 * int(SBhi[b]):8 * int(SBhi[b]) + 8 * Shi])
                et = ep.tile([P, S * TW1], BF16, tag="et")
                qctr = _emit_gathers(nc, et[:], TW1, tbl1[0:LO_END, :],
                                     tbl1[HB:R, :], gxl[:], gxh[:], Slo, Shi, qctr)
                etv = et[:].rearrange("p (s w) -> p s w", w=TW1)

                adb = wp.tile([P, HEADS], BF16, tag="adb")
                nc.vector.tensor_copy(out=adb[:],
                                      in_=adst1[:, b * HEADS:(b + 1) * HEADS])
                al = ep.tile([P, S * HEADS], BF16, tag="al")
                alv = al[:].rearrange("p (s h) -> p s h", h=HEADS)
                nc.vector.tensor_tensor(
                    out=alv, in0=etv[:, :, F1:F1 + HEADS],
                    in1=adb[:].unsqueeze(1).to_broadcast([P, S, HEADS]), op=OP.add)
                lk = ep.tile([P, S * HEADS], BF16, tag="lk")
                nc.scalar.mul(out=lk[:], in_=al[:], mul=NEG_SLOPE)
                nc.vector.tensor_max(out=al[:], in0=al[:], in1=lk[:])
                nc.scalar.activation(out=etv[:, :, F1:F1 + HEADS], in_=alv, func=EXP)
                nc.vector.tensor_tensor(
                    out=etv[:, :, 0:F1].rearrange("p s (h c) -> p s h c", c=HID),
                    in0=etv[:, :, 0:F1].rearrange("p s (h c) -> p s h c", c=HID),
                    in1=etv[:, :, F1:F1 + HEADS].unsqueeze(3)
                        .to_broadcast([P, S, HEADS, HID]),
                    op=OP.mult)
                agg = wp.tile([P, F1 + HEADS], F32, tag="agg")
                nc.vector.reduce_sum(
                    out=agg[:],
                    in_=et[:].rearrange("p (s w) -> p w s", w=TW1)[:, 0:F1 + HEADS, :],
                    axis=AX)
                if debug_outs:
                    nc.sync.dma_start(out=dden1_d[r0:r0 + P, :],
                                      in_=agg[:, F1:F1 + HEADS])
                # epilogue: softmax divide, bias, ELU
                nc.vector.tensor_scalar_max(out=agg[:, F1:F1 + HEADS],
                                            in0=agg[:, F1:F1 + HEADS], scalar1=1e-12)
                rcd = wp.tile([P, HEADS], F32, tag="rcd")
                nc.vector.reciprocal(out=rcd[:], in_=agg[:, F1:F1 + HEADS])
                o1 = wp.tile([P, F1], F32, tag="o1")
                nc.vector.tensor_tensor(
                    out=o1[:].rearrange("p (h c) -> p h c", c=HID),
                    in0=agg[:, 0:F1].rearrange("p (h c) -> p h c", c=HID),
                    in1=rcd[:].unsqueeze(2).to_broadcast([P, HEADS, HID]),
                    op=OP.mult)
                nc.vector.tensor_add(out=o1[:], in0=o1[:], in1=b1sb[:])
                # ELU(x) == max(x, exp(min(x, 0)) - 1)
                ex = wp.tile([P, F1], F32, tag="ex")
                nc.vector.tensor_scalar_min(out=ex[:], in0=o1[:], scalar1=0.0)
                nc.scalar.activation(out=ex[:], in_=ex[:], func=EXP)
                nc.vector.tensor_scalar_add(out=ex[:], in0=ex[:], scalar1=-1.0)
                nc.vector.tensor_max(out=ex[:], in0=ex[:], in1=o1[:])

                # dense layer-2 projection of this block
                tp2 = pp.tile([P, P], F32, tag="tp")
                nc.tensor.transpose(out=tp2[0:F1, :], in_=ex[:], identity=idsb[:])
                o1t = wp.tile([F1, P], F32, tag="o1t")
                nc.scalar.copy(out=o1t[:], in_=tp2[0:F1, :])
                h2p = pp.tile([P, OUT_CH], F32, tag="mm2")
                nc.tensor.matmul(out=h2p[:], lhsT=o1t[:], rhs=w2sb[:],
                                 start=True, stop=True)
                t2 = wp.tile([P, OUT_CH], F32, tag="t2")
                tr2 = wp.tile([P, TW2], F32, tag="tr2")
                nc.vector.tensor_mul(out=t2[:], in0=h2p[:], in1=as2sb[:])
                nc.vector.reduce_sum(out=tr2[:, OUT_CH:OUT_CH + 1], in_=t2[:], axis=AX)
                nc.vector.tensor_mul(out=t2[:], in0=h2p[:], in1=ad2sb[:])
                nc.vector.reduce_sum(out=adst2[:, b:b + 1], in_=t2[:], axis=AX)
                nc.scalar.copy(out=tr2[:, 0:OUT_CH], in_=h2p[:])
                nc.vector.memset(tr2[:, OUT_CH + 1:TW2], 0.0)
                if b == nb - 1:
                    nc.vector.tensor_add(out=tr2[:, OUT_CH:OUT_CH + 1],
                                         in0=tr2[:, OUT_CH:OUT_CH + 1],
                                         in1=sm2sb[:])
                nc.sync.dma_start(out=tbl2_loc[r0:r0 + P, :], in_=tr2[:])

            nc.gpsimd.collective_compute(
                "AllGather", OP.bypass, replica_groups=rg,
                ins=[tbl2_loc[:].opt()], outs=[tbl2[:].opt()])

            # ---- phase C: edge layer 2 -------------------------------------
            qctr = 0
            for b in range(nb):
                r0 = b * P
                Slo, Shi = S_lo[b], S_hi[b]
                S = Slo + Shi
                gxl = ep.tile([P, 8 * Slo], I16, tag="gxl")
                nc.sync.dma_start(out=gxl[:],
                                  in_=glo_d[:, 8 * int(SBlo[b]):8 * int(SBlo[b]) + 8 * Slo])
                gxh = ep.tile([P, 8 * Shi], I16, tag="gxh")
                nc.sync.dma_start(out=gxh[:],
                                  in_=ghi_d[:, 8 * int(SBhi[b]):8 * int(SBhi[b]) + 8 * Shi])
                e2 = ep.tile([P, S * TW2], F32, tag="et")
                qctr = _emit_gathers(nc, e2[:], TW2, tbl2[0:LO_END, :],
                                     tbl2[HB:R, :], gxl[:], gxh[:], Slo, Shi, qctr)
                e2v = e2[:].rearrange("p (s w) -> p s w", w=TW2)

                al2 = ep.tile([P, S], F32, tag="al")
                al2v = al2[:].unsqueeze(2)
                nc.vector.tensor_tensor(
                    out=al2v, in0=e2v[:, :, OUT_CH:OUT_CH + 1],
                    in1=adst2[:, b:b + 1].unsqueeze(1).to_broadcast([P, S, 1]),
                    op=OP.add)
                lk2 = ep.tile([P, S], F32, tag="lk")
                nc.scalar.mul(out=lk2[:], in_=al2[:], mul=NEG_SLOPE)
                nc.vector.tensor_max(out=al2[:], in0=al2[:], in1=lk2[:])
                nc.scalar.activation(out=e2v[:, :, OUT_CH:OUT_CH + 1], in_=al2v,
                                     func=EXP)
                nc.vector.tensor_tensor(
                    out=e2v[:, :, 0:OUT_CH],
                    in0=e2v[:, :, 0:OUT_CH],
                    in1=e2v[:, :, OUT_CH:OUT_CH + 1].to_broadcast([P, S, OUT_CH]),
                    op=OP.mult)
                agg2 = wp.tile([P, OUT_CH + 1], F32, tag="agg2")
                nc.vector.reduce_sum(
                    out=agg2[:],
                    in_=e2[:].rearrange("p (s w) -> p w s", w=TW2)[:, 0:OUT_CH + 1, :],
                    axis=AX)
                nc.vector.tensor_scalar_max(out=agg2[:, OUT_CH:OUT_CH + 1],
                                            in0=agg2[:, OUT_CH:OUT_CH + 1],
                                            scalar1=1e-12)
                rc2 = wp.tile([P, 1], F32, tag="rc2")
                nc.vector.reciprocal(out=rc2[:], in_=agg2[:, OUT_CH:OUT_CH + 1])
                oo = wp.tile([P, OUT_CH], F32, tag="oo")
                nc.vector.tensor_scalar(out=oo[:], in0=agg2[:, 0:OUT_CH],
                                        scalar1=rc2[:], scalar2=None, op0=OP.mult)
                nc.vector.tensor_add(out=oo[:], in0=oo[:], in1=b2sb[:])
                nc.sync.dma_start(out=out_d[r0:r0 + P, :], in_=oo[:])

    nc.compile()
    return nc


_PROGRAM_CACHE = {}


def _in_maps(meta, inputs_rep):
    return [dict(inputs_rep, xg=meta["xg"][c], gilo=meta["gilo"][c],
                 gihi=meta["gihi"][c]) for c in range(NC)]


def _shared_inputs(W1, att_src1, att_dst1, b1, W2, att_src2, att_dst2, b2,
                   F1, OUT_CH):
    rep = lambda v, w: np.broadcast_to(np.asarray(v, np.float32).reshape(1, w),
                                       (P, w)).copy()
    return {
        "w1": np.asarray(W1, np.float32), "as1": rep(att_src1, F1),
        "ad1": rep(att_dst1, F1), "b1r": rep(b1, F1),
        "w2": np.asarray(W2, np.float32), "as2": rep(att_src2, OUT_CH),
        "ad2": rep(att_dst2, OUT_CH), "b2r": rep(b2, OUT_CH),
        "ident": np.eye(P, dtype=np.float32),
        "smask1": _sent_mask(8), "smask2": _sent_mask(1),
    }


def kernel(x, edge_index, W1, att_src1, att_dst1, b1, W2, att_src2, att_dst2, b2):
    global LAST_RESULTS
    x = np.asarray(x, dtype=np.float32)
    edge_index = np.asarray(edge_index)

    IN_CH = x.shape[1]
    HEADS, HID = np.asarray(att_src1).shape
    OUT_CH = np.asarray(W2).shape[1]
    F1 = HEADS * HID

    meta = _preprocess(x, edge_index)

    key = (meta["nppc"], tuple(meta["S_lo"]), tuple(meta["S_hi"]),
           IN_CH, HEADS, HID, OUT_CH)
    if key not in _PROGRAM_CACHE:
        _PROGRAM_CACHE[key] = _build_program(meta, IN_CH, HEADS, HID, OUT_CH)
    nc = _PROGRAM_CACHE[key]

    shared = _shared_inputs(W1, att_src1, att_dst1, b1, W2, att_src2,
                            att_dst2, b2, F1, OUT_CH)
    res = run_bass_kernel_spmd(nc, _in_maps(meta, shared),
                               core_ids=list(range(NC)), trace=TRACE)
    LAST_RESULTS = res

    out_all = np.stack([res.results[c]["out"] for c in range(NC)])
    r = np.arange(meta["N"])
    out_full = np.empty((meta["N"], OUT_CH), dtype=np.float32)
    out_full[meta["order"]] = out_all[r % NC, r // NC]
    return out_full


# revision 18
# speedup vs baseline: 1.0361x; 1.0361x over previous
"""GAT 2-layer (PyG GATConv) model on 8 Trainium2 NeuronCores.

Strategy (graph/data parallel, dst-partitioned):
  * Nodes are sorted by in-degree (desc) and dealt round-robin to the 8
    cores, so block b on every core holds nodes of similar degree.
    Each core owns NPPC local node slots, processed in blocks of 128
    (one SBUF partition lane per destination node).
  * Edges are grouped by destination on the host.  For each block the
    destination node on lane p owns a run of "slots" in the free
    dimension.  One `dma_gather` (InstDMAGatherAnt) per block per table
    half pulls the per-edge source rows [h | a_src] from an AllGather'ed
    node table straight into the [128, S * W] slot grid (the int16 index
    limit forces a lo/hi table split; each lane's slots are split into a
    lo run and a hi run).  Padding slots point at a sentinel table row
    whose a_src = -6e4, which makes exp(leaky_relu(...)) == 0, so pads
    contribute nothing to messages or softmax denominators.
  * The segment softmax + weighted aggregation is dense per-block work:
    alpha = a_src + a_dst (a_dst is resident per-lane), leaky-relu
    (max(x, 0.2x)), exp on the scalar engine, message scale, and one
    strided reduce over the slot axis which also sums the denominators.
  * Layer 1 -> ELU -> layer-2 dense projection happen in the same block
    loop; a second AllGather publishes the layer-2 table; a second edge
    phase produces the output.  Layer-1 table is bf16 (rows padded to
    256B, the dma_gather element granularity); layer-2 table is f32.
"""

import ml_dtypes
import numpy as np

import concourse.bacc as bacc
import concourse.mybir as mybir
import concourse.tile as tile
from concourse.bass_utils import run_bass_kernel_spmd

F32 = mybir.dt.float32
BF16 = mybir.dt.bfloat16
I16 = mybir.dt.int16
AX = mybir.AxisListType.X
OP = mybir.AluOpType
EXP = mybir.ActivationFunctionType.Exp

NC = 8          # cores
P = 128         # partitions / nodes per block
HALF = 32768    # int16 index limit -> lo/hi table split
NEG_SLOPE = 0.2
PAD_ASRC = -60000.0   # sentinel a_src for padding slots: exp(leaky(.)) == 0

LAST_RESULTS = None   # stashed BassKernelResults for test harnesses
TRACE = False         # set True (e.g. from test.py) to capture an NTFF profile


def _ceil_to(x, m):
    return (x + m - 1) // m * m


def _sent_mask(w):
    m = np.zeros((P, w), np.float32)
    m[P - 1, :] = PAD_ASRC
    return m


def _wrap_idx(seg):
    """[NC, 128*S] position-major int16 -> ucode layout [NC, 128, 8*S]
    (idx i at partition i%16, column i//16; replicated across the 8
    16-partition groups)."""
    ncs, n = seg.shape
    w = seg.reshape(ncs, n // 16, 16).transpose(0, 2, 1)   # [NC, 16, cols]
    return np.tile(w, (1, 8, 1)).astype(np.int16)


# --------------------------------------------------------------------------
# host-side graph preprocessing
# --------------------------------------------------------------------------
def _preprocess(x, edge_index):
    N = x.shape[0]
    src = np.concatenate([np.asarray(edge_index[0]), np.arange(N, dtype=np.int64)])
    dst = np.concatenate([np.asarray(edge_index[1]), np.arange(N, dtype=np.int64)])
    src = src.astype(np.int64)
    dst = dst.astype(np.int64)

    deg = np.bincount(dst, minlength=N)
    order = np.argsort(-deg, kind="stable")          # rank -> node id
    rank = np.empty(N, dtype=np.int64)
    rank[order] = np.arange(N)

    core_of = rank % NC
    lid_of = rank // NC                              # local id on its core
    nppc = _ceil_to((N + NC - 1) // NC, P)           # local slots per core
    if nppc * NC <= N:                               # ensure a dummy lane exists
        nppc += P                                    # (hosts the pad sentinel)
    nb = nppc // P                                   # blocks per core
    R = NC * nppc

    gsid = core_of * nppc + lid_of                   # node -> table row
    assert not np.any((core_of == 0) & (lid_of == nppc - 1))
    SENT_LO = nppc - 1                               # core 0's last (dummy) lane
    HI_BASE = HALF if R > HALF else 0                # hi half empty if R fits
    SENT_HI = R - 1 - HI_BASE                        # core NC-1's last lane
    assert R - HI_BASE <= HALF and SENT_HI >= 0

    # per-edge half split and slot position within (node, half)
    gs = gsid[src]
    is_hi = (gs >= HI_BASE).astype(np.int64) if HI_BASE else np.zeros_like(gs)
    key = rank[dst] * 2 + is_hi
    eord = np.argsort(key, kind="stable")
    key_s = key[eord]
    gs_s = gs[eord]
    counts_k = np.bincount(key, minlength=2 * N)
    starts_k = np.concatenate([[0], np.cumsum(counts_k)])[:-1]
    slot = np.arange(key_s.shape[0], dtype=np.int64) - starts_k[key_s]

    k_lo = counts_k[0::2]                            # per rank
    k_hi = counts_k[1::2]
    nwin = nb * NC * P

    def _blockmax(a):
        pad = np.zeros(nwin, dtype=np.int64)
        pad[:N] = a
        return np.maximum(pad.reshape(nb, NC * P).max(1), 1)

    S_lo = _blockmax(k_lo)
    S_hi = _blockmax(k_hi)
    SBlo = np.concatenate([[0], np.cumsum(S_lo)]).astype(np.int64)
    SBhi = np.concatenate([[0], np.cumsum(S_hi)]).astype(np.int64)

    r_s = key_s // 2
    half_s = key_s & 1
    c_e = r_s % NC
    l_e = r_s // NC
    b_e = l_e // P
    p_e = l_e % P
    # position-major flat layout per half: pos = 128*SB[b] + slot*128 + lane
    pos_lo = P * SBlo[b_e] + slot * P + p_e
    pos_hi = P * SBhi[b_e] + slot * P + p_e

    G_lo = np.full((NC, int(P * SBlo[-1])), SENT_LO, dtype=np.int64)
    G_hi = np.full((NC, int(P * SBhi[-1])), SENT_HI, dtype=np.int64)
    lo = half_s == 0
    hi = ~lo
    G_lo[c_e[lo], pos_lo[lo]] = gs_s[lo]
    G_hi[c_e[hi], pos_hi[hi]] = gs_s[hi] - HI_BASE

    gilo = _wrap_idx(G_lo)                            # [NC, 128, 8*sum(S_lo)]
    gihi = _wrap_idx(G_hi)

    xg = np.zeros((NC, nppc, x.shape[1]), dtype=np.float32)
    xg[core_of, lid_of] = np.asarray(x, dtype=np.float32)

    return dict(N=N, nppc=nppc, nb=nb,
                S_lo=[int(v) for v in S_lo], S_hi=[int(v) for v in S_hi],
                SBlo=SBlo, SBhi=SBhi, gilo=gilo, gihi=gihi, HI_BASE=HI_BASE,
                xg=xg, order=order)



GMAX_COLS = 512   # one call per half (single_packet=False packs the ring)


def _emit_gathers(nc, et_ap, tw, tbl_lo_ap, tbl_hi_ap, gxl, gxh, Slo, Shi, qctr):
    """Emit capped dma_gather calls filling slot columns [0,Slo+Shi) of the
    block's slot grid. Returns updated queue counter."""
    for half, (tbl_ap, gx, scnt, base) in enumerate(
            ((tbl_lo_ap, gxl, Slo, 0), (tbl_hi_ap, gxh, Shi, Slo))):
        done = 0
        while done < scnt:
            take = min(GMAX_COLS, scnt - done)
            nc.gpsimd.dma_gather(
                out_ap=et_ap[:, (base + done) * tw:(base + done + take) * tw]
                    .rearrange("p (s w) -> p s w", w=tw),
                in_ap=tbl_ap,
                idxs_ap=gx[:, 8 * done:8 * (done + take)],
                num_idxs=P * take, num_idxs_reg=P * take, elem_size=tw,
                queue_num=0, single_packet=False)
            qctr += 1
            done += take
    return qctr


# --------------------------------------------------------------------------
# device program
# --------------------------------------------------------------------------
def _build_program(meta, IN_CH, HEADS, HID, OUT_CH, debug_outs=False):
    F1 = HEADS * HID                                  # 64
    TW1 = 2 * F1                                      # 128 bf16 = 256B rows
    TW2 = F1                                          # 64 f32  = 256B rows
    nppc, nb = meta["nppc"], meta["nb"]
    S_lo, S_hi, SBlo, SBhi = (meta["S_lo"], meta["S_hi"],
                              meta["SBlo"], meta["SBhi"])
    R = NC * nppc
    HB = meta["HI_BASE"]
    LO_END = min(HALF, R)
    KC = IN_CH // P
    CL = 8 * int(SBlo[-1])                            # idx columns, lo
    CH = 8 * int(SBhi[-1])

    nc = bacc.Bacc("TRN2", target_bir_lowering=False, debug=False,
                   enable_asserts=False, num_devices=NC)

    xg_d = nc.dram_tensor("xg", [nppc, IN_CH], F32, kind="ExternalInput").ap()
    w1_d = nc.dram_tensor("w1", [IN_CH, F1], F32, kind="ExternalInput").ap()
    as1_d = nc.dram_tensor("as1", [P, F1], F32, kind="ExternalInput").ap()
    ad1_d = nc.dram_tensor("ad1", [P, F1], F32, kind="ExternalInput").ap()
    b1_d = nc.dram_tensor("b1r", [P, F1], F32, kind="ExternalInput").ap()
    w2_d = nc.dram_tensor("w2", [F1, OUT_CH], F32, kind="ExternalInput").ap()
    as2_d = nc.dram_tensor("as2", [P, OUT_CH], F32, kind="ExternalInput").ap()
    ad2_d = nc.dram_tensor("ad2", [P, OUT_CH], F32, kind="ExternalInput").ap()
    b2_d = nc.dram_tensor("b2r", [P, OUT_CH], F32, kind="ExternalInput").ap()
    id_d = nc.dram_tensor("ident", [P, P], F32, kind="ExternalInput").ap()
    glo_d = nc.dram_tensor("gilo", [P, CL], I16, kind="ExternalInput").ap()
    ghi_d = nc.dram_tensor("gihi", [P, CH], I16, kind="ExternalInput").ap()
    sm1_d = nc.dram_tensor("smask1", [P, HEADS], F32, kind="ExternalInput").ap()
    sm2_d = nc.dram_tensor("smask2", [P, 1], F32, kind="ExternalInput").ap()
    out_d = nc.dram_tensor("out", [nppc, OUT_CH], F32, kind="ExternalOutput").ap()
    if debug_outs:
        dden1_d = nc.dram_tensor("dden1", [nppc, HEADS], F32, kind="ExternalOutput").ap()

    rg = [list(range(NC))]

    with tile.TileContext(nc) as tc:
        with (tc.tile_pool(name="const", bufs=1) as cp,
              tc.tile_pool(name="work", bufs=3) as wp,
              tc.tile_pool(name="edge", bufs=2) as ep,
              tc.tile_pool(name="psum", bufs=2, space="PSUM") as pp,
              tc.tile_pool(name="dram", bufs=1, space="DRAM") as dp):

            # ---- constants -------------------------------------------------
            w1sb = cp.tile([P, KC * F1], F32)
            nc.sync.dma_start(out=w1sb[:].rearrange("p (i j) -> p i j", j=F1),
                              in_=w1_d.rearrange("(i p) j -> p i j", p=P))
            idsb = cp.tile([P, P], F32)
            nc.sync.dma_start(out=idsb[:], in_=id_d)
            as1sb = cp.tile([P, F1], F32)
            nc.sync.dma_start(out=as1sb[:], in_=as1_d)
            ad1sb = cp.tile([P, F1], F32)
            nc.sync.dma_start(out=ad1sb[:], in_=ad1_d)
            b1sb = cp.tile([P, F1], F32)
            nc.sync.dma_start(out=b1sb[:], in_=b1_d)
            w2sb = cp.tile([F1, OUT_CH], F32)
            nc.sync.dma_start(out=w2sb[:], in_=w2_d)
            as2sb = cp.tile([P, OUT_CH], F32)
            nc.sync.dma_start(out=as2sb[:], in_=as2_d)
            ad2sb = cp.tile([P, OUT_CH], F32)
            nc.sync.dma_start(out=ad2sb[:], in_=ad2_d)
            b2sb = cp.tile([P, OUT_CH], F32)
            nc.sync.dma_start(out=b2sb[:], in_=b2_d)
            sm1sb = cp.tile([P, HEADS], F32)
            nc.sync.dma_start(out=sm1sb[:], in_=sm1_d)
            sm2sb = cp.tile([P, 1], F32)
            nc.sync.dma_start(out=sm2sb[:], in_=sm2_d)

            adst1 = cp.tile([P, nb * HEADS], F32)     # per-block a_dst, layer 1
            adst2 = cp.tile([P, nb], F32)             # per-block a_dst, layer 2

            tbl1_loc = dp.tile([nppc, TW1], BF16)
            tbl1 = dp.tile([R, TW1], BF16)
            tbl2_loc = dp.tile([nppc, TW2], F32)
            tbl2 = dp.tile([R, TW2], F32)

            # ---- phase A: dense layer-1 projection -------------------------
            for b in range(nb):
                r0 = b * P
                xt = wp.tile([P, IN_CH], F32, tag="xt")
                nc.sync.dma_start(out=xt[:], in_=xg_d[r0:r0 + P, :])
                rs = wp.tile([P, 1], F32, tag="rs")
                nc.vector.reduce_sum(out=rs[:], in_=xt[:], axis=AX)
                nc.vector.tensor_scalar_max(out=rs[:], in0=rs[:], scalar1=1e-8)
                rcp = wp.tile([P, 1], F32, tag="rcp")
                nc.vector.reciprocal(out=rcp[:], in_=rs[:])
                nc.vector.tensor_scalar_mul(out=xt[:], in0=xt[:], scalar1=rcp[:])

                hT = wp.tile([P, IN_CH], F32, tag="hT")
                for i in range(KC):
                    tp = pp.tile([P, P], F32, tag="tp")
                    nc.tensor.transpose(out=tp[:], in_=xt[:, i * P:(i + 1) * P],
                                        identity=idsb[:])
                    nc.scalar.copy(out=hT[:, i * P:(i + 1) * P], in_=tp[:])
                h1p = pp.tile([P, F1], F32, tag="mm1")
                for i in range(KC):
                    nc.tensor.matmul(out=h1p[:], lhsT=hT[:, i * P:(i + 1) * P],
                                     rhs=w1sb[:, i * F1:(i + 1) * F1],
                                     start=(i == 0), stop=(i == KC - 1))

                atmp = wp.tile([P, F1], F32, tag="atmp")
                asr = wp.tile([P, HEADS], F32, tag="asr")
                nc.vector.tensor_mul(out=atmp[:], in0=h1p[:], in1=as1sb[:])
                nc.vector.reduce_sum(
                    out=asr[:],
                    in_=atmp[:].rearrange("p (h c) -> p h c", c=HID), axis=AX)
                nc.vector.tensor_mul(out=atmp[:], in0=h1p[:], in1=ad1sb[:])
                nc.vector.reduce_sum(
                    out=adst1[:, b * HEADS:(b + 1) * HEADS],
                    in_=atmp[:].rearrange("p (h c) -> p h c", c=HID), axis=AX)
                if b == nb - 1:
                    # pad-slot sentinel rows ride the AllGather (last dummy
                    # lane of every core; smask1 = PAD_ASRC on lane P-1 only)
                    nc.vector.tensor_add(out=asr[:], in0=asr[:], in1=sm1sb[:])
                trow = wp.tile([P, TW1], BF16, tag="trow")
                nc.scalar.copy(out=trow[:, 0:F1], in_=h1p[:])
                nc.scalar.copy(out=trow[:, F1:F1 + HEADS], in_=asr[:])
                nc.vector.memset(trow[:, F1 + HEADS:TW1], 0.0)
                nc.sync.dma_start(out=tbl1_loc[r0:r0 + P, :], in_=trow[:])

            nc.gpsimd.collective_compute(
                "AllGather", OP.bypass, replica_groups=rg,
                ins=[tbl1_loc[:].opt()], outs=[tbl1[:].opt()])

            # ---- phase B: edge layer 1 + dense layer 2 ---------------------
            qctr = 0
            for b in range(nb):
                r0 = b * P
                Slo, Shi = S_lo[b], S_hi[b]
                S = Slo + Shi
                gxl = ep.tile([P, 8 * Slo], I16, tag="gxl")
                nc.sync.dma_start(out=gxl[:],
                                  in_=glo_d[:, 8 * int(SBlo[b]):8 * int(SBlo[b]) + 8 * Slo])
                gxh = ep.tile([P, 8 * Shi], I16, tag="gxh")
                nc.sync.dma_start(out=gxh[:],
                                  in_=ghi_d[:, 8 * int(SBhi[b]):8 * int(SBhi[b]) + 8 * Shi])
                et = ep.tile([P, S * TW1], BF16, tag="et")
                qctr = _emit_gathers(nc, et[:], TW1, tbl1[0:LO_END, :],
                                     tbl1[HB:R, :], gxl[:], gxh[:], Slo, Shi, qctr)
                etv = et[:].rearrange("p (s w) -> p s w", w=TW1)

                adb = wp.tile([P, HEADS], BF16, tag="adb")
                nc.vector.tensor_copy(out=adb[:],
                                      in_=adst1[:, b * HEADS:(b + 1) * HEADS])
                al = ep.tile([P, S * HEADS], BF16, tag="al")
                alv = al[:].rearrange("p (s h) -> p s h", h=HEADS)
                nc.vector.tensor_tensor(
                    out=alv, in0=etv[:, :, F1:F1 + HEADS],
                    in1=adb[:].unsqueeze(1).to_broadcast([P, S, HEADS]), op=OP.add)
                lk = ep.tile([P, S * HEADS], BF16, tag="lk")
                nc.scalar.mul(out=lk[:], in_=al[:], mul=NEG_SLOPE)
                nc.vector.tensor_max(out=al[:], in0=al[:], in1=lk[:])
                nc.scalar.activation(out=etv[:, :, F1:F1 + HEADS], in_=alv, func=EXP)
                nc.vector.tensor_tensor(
                    out=etv[:, :, 0:F1].rearrange("p s (h c) -> p s h c", c=HID),
                    in0=etv[:, :, 0:F1].rearrange("p s (h c) -> p s h c", c=HID),
                    in1=etv[:, :, F1:F1 + HEADS].unsqueeze(3)
                        .to_broadcast([P, S, HEADS, HID]),
                    op=OP.mult)
                agg = wp.tile([P, F1 + HEADS], F32, tag="agg")
                nc.vector.reduce_sum(
                    out=agg[:],
                    in_=et[:].rearrange("p (s w) -> p w s", w=TW1)[:, 0:F1 + HEADS, :],
                    axis=AX)
                if debug_outs:
                    nc.sync.dma_start(out=dden1_d[r0:r0 + P, :],
                                      in_=agg[:, F1:F1 + HEADS])
                # epilogue: softmax divide, bias, ELU
                nc.vector.tensor_scalar_max(out=agg[:, F1:F1 + HEADS],
                                            in0=agg[:, F1:F1 + HEADS], scalar1=1e-12)
                rcd = wp.tile([P, HEADS], F32, tag="rcd")
                nc.vector.reciprocal(out=rcd[:], in_=agg[:, F1:F1 + HEADS])
                o1 = wp.tile([P, F1], F32, tag="o1")
                nc.vector.tensor_tensor(
                    out=o1[:].rearrange("p (h c) -> p h c", c=HID),
                    in0=agg[:, 0:F1].rearrange("p (h c) -> p h c", c=HID),
                    in1=rcd[:].unsqueeze(2).to_broadcast([P, HEADS, HID]),
                    op=OP.mult)
                nc.vector.tensor_add(out=o1[:], in0=o1[:], in1=b1sb[:])
                # ELU(x) == max(x, exp(min(x, 0)) - 1)
                ex = wp.tile([P, F1], F32, tag="ex")
                nc.vector.tensor_scalar_min(out=ex[:], in0=o1[:], scalar1=0.0)
                nc.scalar.activation(out=ex[:], in_=ex[:], func=EXP)
                nc.vector.tensor_scalar_add(out=ex[:], in0=ex[:], scalar1=-1.0)
                nc.vector.tensor_max(out=ex[:], in0=ex[:], in1=o1[:])

                # dense layer-2 projection of this block
                tp2 = pp.tile([P, P], F32, tag="tp")
                nc.tensor.transpose(out=tp2[0:F1, :], in_=ex[:], identity=idsb[:])
                o1t = wp.tile([F1, P], F32, tag="o1t")
                nc.scalar.copy(out=o1t[:], in_=tp2[0:F1, :])
                h2p = pp.tile([P, OUT_CH], F32, tag="mm2")
                nc.tensor.matmul(out=h2p[:], lhsT=o1t[:], rhs=w2sb[:],
                                 start=True, stop=True)
                t2 = wp.tile([P, OUT_CH], F32, tag="t2")
                tr2 = wp.tile([P, TW2], F32, tag="tr2")
                nc.vector.tensor_mul(out=t2[:], in0=h2p[:], in1=as2sb[:])
                nc.vector.reduce_sum(out=tr2[:, OUT_CH:OUT_CH + 1], in_=t2[:], axis=AX)
                nc.vector.tensor_mul(out=t2[:], in0=h2p[:], in1=ad2sb[:])
                nc.vector.reduce_sum(out=adst2[:, b:b + 1], in_=t2[:], axis=AX)
                nc.scalar.copy(out=tr2[:, 0:OUT_CH], in_=h2p[:])
                nc.vector.memset(tr2[:, OUT_CH + 1:TW2], 0.0)
                if b == nb - 1:
                    nc.vector.tensor_add(out=tr2[:, OUT_CH:OUT_CH + 1],
                                         in0=tr2[:, OUT_CH:OUT_CH + 1],
                                         in1=sm2sb[:])
                nc.sync.dma_start(out=tbl2_loc[r0:r0 + P, :], in_=tr2[:])

            nc.gpsimd.collective_compute(
                "AllGather", OP.bypass, replica_groups=rg,
                ins=[tbl2_loc[:].opt()], outs=[tbl2[:].opt()])

            # ---- phase C: edge layer 2 -------------------------------------
            qctr = 0
            for b in range(nb):
                r0 = b * P
                Slo, Shi = S_lo[b], S_hi[b]
                S = Slo + Shi
                gxl = ep.tile([P, 8 * Slo], I16, tag="gxl")
                nc.sync.dma_start(out=gxl[:],
                                  in_=glo_d[:, 8 * int(SBlo[b]):8 * int(SBlo[b]) + 8 * Slo])
                gxh = ep.tile([P, 8 * Shi], I16, tag="gxh")
                nc.sync.dma_start(out=gxh[:],
                                  in_=ghi_d[:, 8 * int(SBhi[b]):8 * int(SBhi[b]) + 8 * Shi])
                e2 = ep.tile([P, S * TW2], F32, tag="et")
                qctr = _emit_gathers(nc, e2[:], TW2, tbl2[0:LO_END, :],
                                     tbl2[HB:R, :], gxl[:], gxh[:], Slo, Shi, qctr)
                e2v = e2[:].rearrange("p (s w) -> p s w", w=TW2)

                al2 = ep.tile([P, S], F32, tag="al")
                al2v = al2[:].unsqueeze(2)
                nc.vector.tensor_tensor(
                    out=al2v, in0=e2v[:, :, OUT_CH:OUT_CH + 1],
                    in1=adst2[:, b:b + 1].unsqueeze(1).to_broadcast([P, S, 1]),
                    op=OP.add)
                lk2 = ep.tile([P, S], F32, tag="lk")
                nc.scalar.mul(out=lk2[:], in_=al2[:], mul=NEG_SLOPE)
                nc.vector.tensor_max(out=al2[:], in0=al2[:], in1=lk2[:])
                nc.scalar.activation(out=e2v[:, :, OUT_CH:OUT_CH + 1], in_=al2v,
                                     func=EXP)
                nc.vector.tensor_tensor(
                    out=e2v[:, :, 0:OUT_CH],
                    in0=e2v[:, :, 0:OUT_CH],
                    in1=e2v[:, :, OUT_CH:OUT_CH + 1].to_broadcast([P, S, OUT_CH]),
                    op=OP.mult)
                agg2 = wp.tile([P, OUT_CH + 1], F32, tag="agg2")
                nc.vector.reduce_sum(
                    out=agg2[:],
                    in_=e2[:].rearrange("p (s w) -> p w s", w=TW2)[:, 0:OUT_CH + 1, :],
                    axis=AX)
                nc.vector.tensor_scalar_max(out=agg2[:, OUT_CH:OUT_CH + 1],
                                            in0=agg2[:, OUT_CH:OUT_CH + 1],
                                            scalar1=1e-12)
                rc2 = wp.tile([P, 1], F32, tag="rc2")
                nc.vector.reciprocal(out=rc2[:], in_=agg2[:, OUT_CH:OUT_CH + 1])
                oo = wp.tile([P, OUT_CH], F32, tag="oo")
                nc.vector.tensor_scalar(out=oo[:], in0=agg2[:, 0:OUT_CH],
                                        scalar1=rc2[:], scalar2=None, op0=OP.mult)
                nc.vector.tensor_add(out=oo[:], in0=oo[:], in1=b2sb[:])
                nc.sync.dma_start(out=out_d[r0:r0 + P, :], in_=oo[:])

    nc.compile()
    return nc


_PROGRAM_CACHE = {}


def _in_maps(meta, inputs_rep):
    return [dict(inputs_rep, xg=meta["xg"][c], gilo=meta["gilo"][c],
                 gihi=meta["gihi"][c]) for c in range(NC)]


def _shared_inputs(W1, att_src1, att_dst1, b1, W2, att_src2, att_dst2, b2,
                   F1, OUT_CH):
    rep = lambda v, w: np.broadcast_to(np.asarray(v, np.float32).reshape(1, w),
                                       (P, w)).copy()
    return {
        "w1": np.asarray(W1, np.float32), "as1": rep(att_src1, F1),
        "ad1": rep(att_dst1, F1), "b1r": rep(b1, F1),
        "w2": np.asarray(W2, np.float32), "as2": rep(att_src2, OUT_CH),
        "ad2": rep(att_dst2, OUT_CH), "b2r": rep(b2, OUT_CH),
        "ident": np.eye(P, dtype=np.float32),
        "smask1": _sent_mask(8), "smask2": _sent_mask(1),
    }


def kernel(x, edge_index, W1, att_src1, att_dst1, b1, W2, att_src2, att_dst2, b2):
    global LAST_RESULTS
    x = np.asarray(x, dtype=np.float32)
    edge_index = np.asarray(edge_index)

    IN_CH = x.shape[1]
    HEADS, HID = np.asarray(att_src1).shape
    OUT_CH = np.asarray(W2).shape[1]
    F1 = HEADS * HID

    meta = _preprocess(x, edge_index)

    key = (meta["nppc"], tuple(meta["S_lo"]), tuple(meta["S_hi"]),
           IN_CH, HEADS, HID, OUT_CH)
    if key not in _PROGRAM_CACHE:
        _PROGRAM_CACHE[key] = _build_program(meta, IN_CH, HEADS, HID, OUT_CH)
    nc = _PROGRAM_CACHE[key]

    shared = _shared_inputs(W1, att_src1, att_dst1, b1, W2, att_src2,
                            att_dst2, b2, F1, OUT_CH)
    res = run_bass_kernel_spmd(nc, _in_maps(meta, shared),
                               core_ids=list(range(NC)), trace=TRACE)
    LAST_RESULTS = res

    out_all = np.stack([res.results[c]["out"] for c in range(NC)])
    r = np.arange(meta["N"])
    out_full = np.empty((meta["N"], OUT_CH), dtype=np.float32)
    out_full[meta["order"]] = out_all[r % NC, r // NC]
    return out_full


# revision 19
# speedup vs baseline: 1.2435x; 1.2002x over previous
"""GAT 2-layer (PyG GATConv) model on 8 Trainium2 NeuronCores.

Strategy (graph/data parallel, dst-partitioned):
  * Nodes are sorted by in-degree (desc) and dealt round-robin to the 8
    cores, so block b on every core holds nodes of similar degree.
    Each core owns NPPC local node slots, processed in blocks of 128
    (one SBUF partition lane per destination node).
  * Edges are grouped by destination on the host.  For each block the
    destination node on lane p owns a run of "slots" in the free
    dimension.  One `dma_gather` (InstDMAGatherAnt) per block per table
    half pulls the per-edge source rows [h | a_src] from an AllGather'ed
    node table straight into the [128, S * W] slot grid (the int16 index
    limit forces a lo/hi table split; each lane's slots are split into a
    lo run and a hi run).  Padding slots point at a sentinel table row
    whose a_src = -6e4, which makes exp(leaky_relu(...)) == 0, so pads
    contribute nothing to messages or softmax denominators.
  * The segment softmax + weighted aggregation is dense per-block work:
    alpha = a_src + a_dst (a_dst is resident per-lane), leaky-relu
    (max(x, 0.2x)), exp on the scalar engine, message scale, and one
    strided reduce over the slot axis which also sums the denominators.
  * Layer 1 -> ELU -> layer-2 dense projection happen in the same block
    loop; a second AllGather publishes the layer-2 table; a second edge
    phase produces the output.  Layer-1 table is bf16 (rows padded to
    256B, the dma_gather element granularity); layer-2 table is f32.
"""

import ml_dtypes
import numpy as np

import concourse.bacc as bacc
import concourse.mybir as mybir
import concourse.tile as tile
from concourse.bass_utils import run_bass_kernel_spmd

F32 = mybir.dt.float32
BF16 = mybir.dt.bfloat16
I16 = mybir.dt.int16
AX = mybir.AxisListType.X
OP = mybir.AluOpType
EXP = mybir.ActivationFunctionType.Exp

NC = 8          # cores
P = 128         # partitions / nodes per block
HALF = 32768    # int16 index limit -> lo/hi table split
NEG_SLOPE = 0.2
PAD_ASRC = -60000.0   # sentinel a_src for padding slots: exp(leaky(.)) == 0

LAST_RESULTS = None   # stashed BassKernelResults for test harnesses
TRACE = False         # set True (e.g. from test.py) to capture an NTFF profile


def _ceil_to(x, m):
    return (x + m - 1) // m * m


def _sent_mask(w):
    m = np.zeros((P, w), np.float32)
    m[P - 1, :] = PAD_ASRC
    return m


def _wrap_idx(seg):
    """[NC, 128*S] position-major int16 -> ucode layout [NC, 128, 8*S]
    (idx i at partition i%16, column i//16; replicated across the 8
    16-partition groups)."""
    ncs, n = seg.shape
    w = seg.reshape(ncs, n // 16, 16).transpose(0, 2, 1)   # [NC, 16, cols]
    return np.tile(w, (1, 8, 1)).astype(np.int16)


# --------------------------------------------------------------------------
# host-side graph preprocessing
# --------------------------------------------------------------------------
def _preprocess(x, edge_index):
    N = x.shape[0]
    src = np.concatenate([np.asarray(edge_index[0]), np.arange(N, dtype=np.int64)])
    dst = np.concatenate([np.asarray(edge_index[1]), np.arange(N, dtype=np.int64)])
    src = src.astype(np.int64)
    dst = dst.astype(np.int64)

    deg = np.bincount(dst, minlength=N)
    order = np.argsort(-deg, kind="stable")          # rank -> node id
    rank = np.empty(N, dtype=np.int64)
    rank[order] = np.arange(N)

    core_of = rank % NC
    lid_of = rank // NC                              # local id on its core
    nppc = _ceil_to((N + NC - 1) // NC, P)           # local slots per core
    if nppc * NC <= N:                               # ensure a dummy lane exists
        nppc += P                                    # (hosts the pad sentinel)
    nb = nppc // P                                   # blocks per core
    R = NC * nppc

    gsid = core_of * nppc + lid_of                   # node -> table row
    assert not np.any((core_of == 0) & (lid_of == nppc - 1))
    SENT_LO = nppc - 1                               # core 0's last (dummy) lane
    HI_BASE = HALF if R > HALF else 0                # hi half empty if R fits
    SENT_HI = R - 1 - HI_BASE                        # core NC-1's last lane
    assert R - HI_BASE <= HALF and SENT_HI >= 0

    # per-edge half split and slot position within (node, half)
    gs = gsid[src]
    is_hi = (gs >= HI_BASE).astype(np.int64) if HI_BASE else np.zeros_like(gs)
    key = rank[dst] * 2 + is_hi
    eord = np.argsort(key, kind="stable")
    key_s = key[eord]
    gs_s = gs[eord]
    counts_k = np.bincount(key, minlength=2 * N)
    starts_k = np.concatenate([[0], np.cumsum(counts_k)])[:-1]
    slot = np.arange(key_s.shape[0], dtype=np.int64) - starts_k[key_s]

    k_lo = counts_k[0::2]                            # per rank
    k_hi = counts_k[1::2]
    nwin = nb * NC * P

    def _blockmax(a):
        pad = np.zeros(nwin, dtype=np.int64)
        pad[:N] = a
        return np.maximum(pad.reshape(nb, NC * P).max(1), 1)

    S_lo = _blockmax(k_lo)
    S_hi = _blockmax(k_hi)
    SBlo = np.concatenate([[0], np.cumsum(S_lo)]).astype(np.int64)
    SBhi = np.concatenate([[0], np.cumsum(S_hi)]).astype(np.int64)

    r_s = key_s // 2
    half_s = key_s & 1
    c_e = r_s % NC
    l_e = r_s // NC
    b_e = l_e // P
    p_e = l_e % P
    # position-major flat layout per half: pos = 128*SB[b] + slot*128 + lane
    pos_lo = P * SBlo[b_e] + slot * P + p_e
    pos_hi = P * SBhi[b_e] + slot * P + p_e

    G_lo = np.full((NC, int(P * SBlo[-1])), SENT_LO, dtype=np.int64)
    G_hi = np.full((NC, int(P * SBhi[-1])), SENT_HI, dtype=np.int64)
    lo = half_s == 0
    hi = ~lo
    G_lo[c_e[lo], pos_lo[lo]] = gs_s[lo]
    G_hi[c_e[hi], pos_hi[hi]] = gs_s[hi] - HI_BASE

    gilo = _wrap_idx(G_lo)                            # [NC, 128, 8*sum(S_lo)]
    gihi = _wrap_idx(G_hi)

    xg = np.zeros((NC, nppc, x.shape[1]), dtype=np.float32)
    xg[core_of, lid_of] = np.asarray(x, dtype=np.float32)

    return dict(N=N, nppc=nppc, nb=nb,
                S_lo=[int(v) for v in S_lo], S_hi=[int(v) for v in S_hi],
                SBlo=SBlo, SBhi=SBhi, gilo=gilo, gihi=gihi, HI_BASE=HI_BASE,
                xg=xg, order=order)



GMAX_COLS = 512   # one call per half (single_packet=False packs the ring)


def _emit_gathers(nc, et_ap, tw, tbl_lo_ap, tbl_hi_ap, gxl, gxh, Slo, Shi, qctr):
    """Emit capped dma_gather calls filling slot columns [0,Slo+Shi) of the
    block's slot grid. Returns updated queue counter."""
    for half, (tbl_ap, gx, scnt, base) in enumerate(
            ((tbl_lo_ap, gxl, Slo, 0), (tbl_hi_ap, gxh, Shi, Slo))):
        done = 0
        while done < scnt:
            take = min(GMAX_COLS, scnt - done)
            nc.gpsimd.dma_gather(
                out_ap=et_ap[:, (base + done) * tw:(base + done + take) * tw]
                    .rearrange("p (s w) -> p s w", w=tw),
                in_ap=tbl_ap,
                idxs_ap=gx[:, 8 * done:8 * (done + take)],
                num_idxs=P * take, num_idxs_reg=P * take, elem_size=tw,
                queue_num=0, single_packet=False)
            qctr += 1
            done += take
    return qctr


# --------------------------------------------------------------------------
# device program
# --------------------------------------------------------------------------
def _build_program(meta, IN_CH, HEADS, HID, OUT_CH, debug_outs=False):
    F1 = HEADS * HID                                  # 64
    TW1 = 2 * F1                                      # 128 bf16 = 256B rows
    TW2 = F1                                          # 64 f32  = 256B rows
    nppc, nb = meta["nppc"], meta["nb"]
    S_lo, S_hi, SBlo, SBhi = (meta["S_lo"], meta["S_hi"],
                              meta["SBlo"], meta["SBhi"])
    R = NC * nppc
    HB = meta["HI_BASE"]
    LO_END = min(HALF, R)
    KC = IN_CH // P
    CL = 8 * int(SBlo[-1])                            # idx columns, lo
    CH = 8 * int(SBhi[-1])

    nc = bacc.Bacc("TRN2", target_bir_lowering=False, debug=False,
                   enable_asserts=False, num_devices=NC)

    xg_d = nc.dram_tensor("xg", [nppc, IN_CH], F32, kind="ExternalInput").ap()
    w1_d = nc.dram_tensor("w1", [IN_CH, F1], F32, kind="ExternalInput").ap()
    as1_d = nc.dram_tensor("as1", [P, F1], F32, kind="ExternalInput").ap()
    ad1_d = nc.dram_tensor("ad1", [P, F1], F32, kind="ExternalInput").ap()
    b1_d = nc.dram_tensor("b1r", [P, F1], F32, kind="ExternalInput").ap()
    w2_d = nc.dram_tensor("w2", [F1, OUT_CH], F32, kind="ExternalInput").ap()
    as2_d = nc.dram_tensor("as2", [P, OUT_CH], F32, kind="ExternalInput").ap()
    ad2_d = nc.dram_tensor("ad2", [P, OUT_CH], F32, kind="ExternalInput").ap()
    b2_d = nc.dram_tensor("b2r", [P, OUT_CH], F32, kind="ExternalInput").ap()
    id_d = nc.dram_tensor("ident", [P, P], F32, kind="ExternalInput").ap()
    glo_d = nc.dram_tensor("gilo", [P, CL], I16, kind="ExternalInput").ap()
    ghi_d = nc.dram_tensor("gihi", [P, CH], I16, kind="ExternalInput").ap()
    sm1_d = nc.dram_tensor("smask1", [P, HEADS], F32, kind="ExternalInput").ap()
    sm2_d = nc.dram_tensor("smask2", [P, 1], F32, kind="ExternalInput").ap()
    out_d = nc.dram_tensor("out", [nppc, OUT_CH], F32, kind="ExternalOutput").ap()
    if debug_outs:
        dden1_d = nc.dram_tensor("dden1", [nppc, HEADS], F32, kind="ExternalOutput").ap()

    rg = [list(range(NC))]

    with tile.TileContext(nc) as tc:
        with (tc.tile_pool(name="const", bufs=1) as cp,
              tc.tile_pool(name="work", bufs=3) as wp,
              tc.tile_pool(name="edge", bufs=3) as ep,
              tc.tile_pool(name="psum", bufs=2, space="PSUM") as pp,
              tc.tile_pool(name="dram", bufs=1, space="DRAM") as dp):

            # ---- constants -------------------------------------------------
            w1sb = cp.tile([P, KC * F1], F32)
            nc.sync.dma_start(out=w1sb[:].rearrange("p (i j) -> p i j", j=F1),
                              in_=w1_d.rearrange("(i p) j -> p i j", p=P))
            idsb = cp.tile([P, P], F32)
            nc.sync.dma_start(out=idsb[:], in_=id_d)
            as1sb = cp.tile([P, F1], F32)
            nc.sync.dma_start(out=as1sb[:], in_=as1_d)
            ad1sb = cp.tile([P, F1], F32)
            nc.sync.dma_start(out=ad1sb[:], in_=ad1_d)
            b1sb = cp.tile([P, F1], F32)
            nc.sync.dma_start(out=b1sb[:], in_=b1_d)
            w2sb = cp.tile([F1, OUT_CH], F32)
            nc.sync.dma_start(out=w2sb[:], in_=w2_d)
            as2sb = cp.tile([P, OUT_CH], F32)
            nc.sync.dma_start(out=as2sb[:], in_=as2_d)
            ad2sb = cp.tile([P, OUT_CH], F32)
            nc.sync.dma_start(out=ad2sb[:], in_=ad2_d)
            b2sb = cp.tile([P, OUT_CH], F32)
            nc.sync.dma_start(out=b2sb[:], in_=b2_d)
            sm1sb = cp.tile([P, HEADS], F32)
            nc.sync.dma_start(out=sm1sb[:], in_=sm1_d)
            sm2sb = cp.tile([P, 1], F32)
            nc.sync.dma_start(out=sm2sb[:], in_=sm2_d)

            adst1 = cp.tile([P, nb * HEADS], F32)     # per-block a_dst, layer 1
            adst2 = cp.tile([P, nb], F32)             # per-block a_dst, layer 2

            tbl1_loc = dp.tile([nppc, TW1], BF16)
            tbl1 = dp.tile([R, TW1], BF16)
            tbl2_loc = dp.tile([nppc, TW2], F32)
            tbl2 = dp.tile([R, TW2], F32)

            # ---- phase A: dense layer-1 projection -------------------------
            for b in range(nb):
                r0 = b * P
                xt = wp.tile([P, IN_CH], F32, tag="xt")
                nc.sync.dma_start(out=xt[:], in_=xg_d[r0:r0 + P, :])
                rs = wp.tile([P, 1], F32, tag="rs")
                nc.vector.reduce_sum(out=rs[:], in_=xt[:], axis=AX)
                nc.vector.tensor_scalar_max(out=rs[:], in0=rs[:], scalar1=1e-8)
                rcp = wp.tile([P, 1], F32, tag="rcp")
                nc.vector.reciprocal(out=rcp[:], in_=rs[:])
                nc.vector.tensor_scalar_mul(out=xt[:], in0=xt[:], scalar1=rcp[:])

                hT = wp.tile([P, IN_CH], F32, tag="hT")
                for i in range(KC):
                    tp = pp.tile([P, P], F32, tag="tp")
                    nc.tensor.transpose(out=tp[:], in_=xt[:, i * P:(i + 1) * P],
                                        identity=idsb[:])
                    nc.scalar.copy(out=hT[:, i * P:(i + 1) * P], in_=tp[:])
                h1p = pp.tile([P, F1], F32, tag="mm1")
                for i in range(KC):
                    nc.tensor.matmul(out=h1p[:], lhsT=hT[:, i * P:(i + 1) * P],
                                     rhs=w1sb[:, i * F1:(i + 1) * F1],
                                     start=(i == 0), stop=(i == KC - 1))

                atmp = wp.tile([P, F1], F32, tag="atmp")
                asr = wp.tile([P, HEADS], F32, tag="asr")
                nc.vector.tensor_mul(out=atmp[:], in0=h1p[:], in1=as1sb[:])
                nc.vector.reduce_sum(
                    out=asr[:],
                    in_=atmp[:].rearrange("p (h c) -> p h c", c=HID), axis=AX)
                nc.vector.tensor_mul(out=atmp[:], in0=h1p[:], in1=ad1sb[:])
                nc.vector.reduce_sum(
                    out=adst1[:, b * HEADS:(b + 1) * HEADS],
                    in_=atmp[:].rearrange("p (h c) -> p h c", c=HID), axis=AX)
                if b == nb - 1:
                    # pad-slot sentinel rows ride the AllGather (last dummy
                    # lane of every core; smask1 = PAD_ASRC on lane P-1 only)
                    nc.vector.tensor_add(out=asr[:], in0=asr[:], in1=sm1sb[:])
                trow = wp.tile([P, TW1], BF16, tag="trow")
                nc.scalar.copy(out=trow[:, 0:F1], in_=h1p[:])
                nc.scalar.copy(out=trow[:, F1:F1 + HEADS], in_=asr[:])
                nc.vector.memset(trow[:, F1 + HEADS:TW1], 0.0)
                nc.sync.dma_start(out=tbl1_loc[r0:r0 + P, :], in_=trow[:])

            nc.gpsimd.collective_compute(
                "AllGather", OP.bypass, replica_groups=rg,
                ins=[tbl1_loc[:].opt()], outs=[tbl1[:].opt()])

            # ---- phase B: edge layer 1 + dense layer 2 ---------------------
            qctr = 0
            for b in range(nb):
                r0 = b * P
                Slo, Shi = S_lo[b], S_hi[b]
                S = Slo + Shi
                gxl = ep.tile([P, 8 * Slo], I16, tag="gxl")
                nc.sync.dma_start(out=gxl[:],
                                  in_=glo_d[:, 8 * int(SBlo[b]):8 * int(SBlo[b]) + 8 * Slo])
                gxh = ep.tile([P, 8 * Shi], I16, tag="gxh")
                nc.sync.dma_start(out=gxh[:],
                                  in_=ghi_d[:, 8 * int(SBhi[b]):8 * int(SBhi[b]) + 8 * Shi])
                et = ep.tile([P, S * TW1], BF16, tag="et")
                qctr = _emit_gathers(nc, et[:], TW1, tbl1[0:LO_END, :],
                                     tbl1[HB:R, :], gxl[:], gxh[:], Slo, Shi, qctr)
                etv = et[:].rearrange("p (s w) -> p s w", w=TW1)

                adb = wp.tile([P, HEADS], BF16, tag="adb")
                nc.vector.tensor_copy(out=adb[:],
                                      in_=adst1[:, b * HEADS:(b + 1) * HEADS])
                al = ep.tile([P, S * HEADS], BF16, tag="al")
                alv = al[:].rearrange("p (s h) -> p s h", h=HEADS)
                nc.vector.tensor_tensor(
                    out=alv, in0=etv[:, :, F1:F1 + HEADS],
                    in1=adb[:].unsqueeze(1).to_broadcast([P, S, HEADS]), op=OP.add)
                lk = ep.tile([P, S * HEADS], BF16, tag="lk")
                nc.scalar.mul(out=lk[:], in_=al[:], mul=NEG_SLOPE)
                nc.vector.tensor_max(out=al[:], in0=al[:], in1=lk[:])
                nc.scalar.activation(out=etv[:, :, F1:F1 + HEADS], in_=alv, func=EXP)
                nc.vector.tensor_tensor(
                    out=etv[:, :, 0:F1].rearrange("p s (h c) -> p s h c", c=HID),
                    in0=etv[:, :, 0:F1].rearrange("p s (h c) -> p s h c", c=HID),
                    in1=etv[:, :, F1:F1 + HEADS].unsqueeze(3)
                        .to_broadcast([P, S, HEADS, HID]),
                    op=OP.mult)
                agg = wp.tile([P, F1 + HEADS], F32, tag="agg")
                nc.vector.reduce_sum(
                    out=agg[:],
                    in_=et[:].rearrange("p (s w) -> p w s", w=TW1)[:, 0:F1 + HEADS, :],
                    axis=AX)
                if debug_outs:
                    nc.sync.dma_start(out=dden1_d[r0:r0 + P, :],
                                      in_=agg[:, F1:F1 + HEADS])
                # epilogue: softmax divide, bias, ELU
                nc.vector.tensor_scalar_max(out=agg[:, F1:F1 + HEADS],
                                            in0=agg[:, F1:F1 + HEADS], scalar1=1e-12)
                rcd = wp.tile([P, HEADS], F32, tag="rcd")
                nc.vector.reciprocal(out=rcd[:], in_=agg[:, F1:F1 + HEADS])
                o1 = wp.tile([P, F1], F32, tag="o1")
                nc.vector.tensor_tensor(
                    out=o1[:].rearrange("p (h c) -> p h c", c=HID),
                    in0=agg[:, 0:F1].rearrange("p (h c) -> p h c", c=HID),
                    in1=rcd[:].unsqueeze(2).to_broadcast([P, HEADS, HID]),
                    op=OP.mult)
                nc.vector.tensor_add(out=o1[:], in0=o1[:], in1=b1sb[:])
                # ELU(x) == max(x, exp(min(x, 0)) - 1)
                ex = wp.tile([P, F1], F32, tag="ex")
                nc.vector.tensor_scalar_min(out=ex[:], in0=o1[:], scalar1=0.0)
                nc.scalar.activation(out=ex[:], in_=ex[:], func=EXP)
                nc.vector.tensor_scalar_add(out=ex[:], in0=ex[:], scalar1=-1.0)
                nc.vector.tensor_max(out=ex[:], in0=ex[:], in1=o1[:])

                # dense layer-2 projection of this block
                tp2 = pp.tile([P, P], F32, tag="tp")
                nc.tensor.transpose(out=tp2[0:F1, :], in_=ex[:], identity=idsb[:])
                o1t = wp.tile([F1, P], F32, tag="o1t")
                nc.scalar.copy(out=o1t[:], in_=tp2[0:F1, :])
                h2p = pp.tile([P, OUT_CH], F32, tag="mm2")
                nc.tensor.matmul(out=h2p[:], lhsT=o1t[:], rhs=w2sb[:],
                                 start=True, stop=True)
                t2 = wp.tile([P, OUT_CH], F32, tag="t2")
                tr2 = wp.tile([P, TW2], F32, tag="tr2")
                nc.vector.tensor_mul(out=t2[:], in0=h2p[:], in1=as2sb[:])
                nc.vector.reduce_sum(out=tr2[:, OUT_CH:OUT_CH + 1], in_=t2[:], axis=AX)
                nc.vector.tensor_mul(out=t2[:], in0=h2p[:], in1=ad2sb[:])
                nc.vector.reduce_sum(out=adst2[:, b:b + 1], in_=t2[:], axis=AX)
                nc.scalar.copy(out=tr2[:, 0:OUT_CH], in_=h2p[:])
                nc.vector.memset(tr2[:, OUT_CH + 1:TW2], 0.0)
                if b == nb - 1:
                    nc.vector.tensor_add(out=tr2[:, OUT_CH:OUT_CH + 1],
                                         in0=tr2[:, OUT_CH:OUT_CH + 1],
                                         in1=sm2sb[:])
                nc.sync.dma_start(out=tbl2_loc[r0:r0 + P, :], in_=tr2[:])

            nc.gpsimd.collective_compute(
                "AllGather", OP.bypass, replica_groups=rg,
                ins=[tbl2_loc[:].opt()], outs=[tbl2[:].opt()])

            # ---- phase C: edge layer 2 -------------------------------------
            qctr = 0
            for b in range(nb):
                r0 = b * P
                Slo, Shi = S_lo[b], S_hi[b]
                S = Slo + Shi
                gxl = ep.tile([P, 8 * Slo], I16, tag="gxl")
                nc.sync.dma_start(out=gxl[:],
                                  in_=glo_d[:, 8 * int(SBlo[b]):8 * int(SBlo[b]) + 8 * Slo])
                gxh = ep.tile([P, 8 * Shi], I16, tag="gxh")
                nc.sync.dma_start(out=gxh[:],
                                  in_=ghi_d[:, 8 * int(SBhi[b]):8 * int(SBhi[b]) + 8 * Shi])
                e2 = ep.tile([P, S * TW2], F32, tag="et")
                qctr = _emit_gathers(nc, e2[:], TW2, tbl2[0:LO_END, :],
                                     tbl2[HB:R, :], gxl[:], gxh[:], Slo, Shi, qctr)
                e2v = e2[:].rearrange("p (s w) -> p s w", w=TW2)

                al2 = ep.tile([P, S], F32, tag="al")
                al2v = al2[:].unsqueeze(2)
                nc.vector.tensor_tensor(
                    out=al2v, in0=e2v[:, :, OUT_CH:OUT_CH + 1],
                    in1=adst2[:, b:b + 1].unsqueeze(1).to_broadcast([P, S, 1]),
                    op=OP.add)
                lk2 = ep.tile([P, S], F32, tag="lk")
                nc.scalar.mul(out=lk2[:], in_=al2[:], mul=NEG_SLOPE)
                nc.vector.tensor_max(out=al2[:], in0=al2[:], in1=lk2[:])
                nc.scalar.activation(out=e2v[:, :, OUT_CH:OUT_CH + 1], in_=al2v,
                                     func=EXP)
                nc.vector.tensor_tensor(
                    out=e2v[:, :, 0:OUT_CH],
                    in0=e2v[:, :, 0:OUT_CH],
                    in1=e2v[:, :, OUT_CH:OUT_CH + 1].to_broadcast([P, S, OUT_CH]),
                    op=OP.mult)
                agg2 = wp.tile([P, OUT_CH + 1], F32, tag="agg2")
                nc.vector.reduce_sum(
                    out=agg2[:],
                    in_=e2[:].rearrange("p (s w) -> p w s", w=TW2)[:, 0:OUT_CH + 1, :],
                    axis=AX)
                nc.vector.tensor_scalar_max(out=agg2[:, OUT_CH:OUT_CH + 1],
                                            in0=agg2[:, OUT_CH:OUT_CH + 1],
                                            scalar1=1e-12)
                rc2 = wp.tile([P, 1], F32, tag="rc2")
                nc.vector.reciprocal(out=rc2[:], in_=agg2[:, OUT_CH:OUT_CH + 1])
                oo = wp.tile([P, OUT_CH], F32, tag="oo")
                nc.vector.tensor_scalar(out=oo[:], in0=agg2[:, 0:OUT_CH],
                                        scalar1=rc2[:], scalar2=None, op0=OP.mult)
                nc.vector.tensor_add(out=oo[:], in0=oo[:], in1=b2sb[:])
                nc.sync.dma_start(out=out_d[r0:r0 + P, :], in_=oo[:])

    nc.compile()
    return nc


_PROGRAM_CACHE = {}


def _in_maps(meta, inputs_rep):
    return [dict(inputs_rep, xg=meta["xg"][c], gilo=meta["gilo"][c],
                 gihi=meta["gihi"][c]) for c in range(NC)]


def _shared_inputs(W1, att_src1, att_dst1, b1, W2, att_src2, att_dst2, b2,
                   F1, OUT_CH):
    rep = lambda v, w: np.broadcast_to(np.asarray(v, np.float32).reshape(1, w),
                                       (P, w)).copy()
    return {
        "w1": np.asarray(W1, np.float32), "as1": rep(att_src1, F1),
        "ad1": rep(att_dst1, F1), "b1r": rep(b1, F1),
        "w2": np.asarray(W2, np.float32), "as2": rep(att_src2, OUT_CH),
        "ad2": rep(att_dst2, OUT_CH), "b2r": rep(b2, OUT_CH),
        "ident": np.eye(P, dtype=np.float32),
        "smask1": _sent_mask(8), "smask2": _sent_mask(1),
    }


def kernel(x, edge_index, W1, att_src1, att_dst1, b1, W2, att_src2, att_dst2, b2):
    global LAST_RESULTS
    x = np.asarray(x, dtype=np.float32)
    edge_index = np.asarray(edge_index)

    IN_CH = x.shape[1]
    HEADS, HID = np.asarray(att_src1).shape
    OUT_CH = np.asarray(W2).shape[1]
    F1 = HEADS * HID

    meta = _preprocess(x, edge_index)

    key = (meta["nppc"], tuple(meta["S_lo"]), tuple(meta["S_hi"]),
           IN_CH, HEADS, HID, OUT_CH)
    if key not in _PROGRAM_CACHE:
        _PROGRAM_CACHE[key] = _build_program(meta, IN_CH, HEADS, HID, OUT_CH)
    nc = _PROGRAM_CACHE[key]

    shared = _shared_inputs(W1, att_src1, att_dst1, b1, W2, att_src2,
                            att_dst2, b2, F1, OUT_CH)
    res = run_bass_kernel_spmd(nc, _in_maps(meta, shared),
                               core_ids=list(range(NC)), trace=TRACE)
    LAST_RESULTS = res

    out_all = np.stack([res.results[c]["out"] for c in range(NC)])
    r = np.arange(meta["N"])
    out_full = np.empty((meta["N"], OUT_CH), dtype=np.float32)
    out_full[meta["order"]] = out_all[r % NC, r // NC]
    return out_full
